# revision 1
# baseline (speedup 1.0000x reference)
"""Causal self-attention Bass/Tile kernel for 8 Trainium2 NeuronCores.

Problem: B=4, T=2048, C=1024, H=16, D=64 (fp32).
  qkv = x @ w_qkv + b_qkv ; causal softmax attention ; y @ w_out + b_out

Sharding (8 cores): core i handles batch b = i//2 and head-group hg = i%2
(8 of the 16 heads). Each core computes Q/K/V projections restricted to its
heads, full causal attention for those heads, and a partial output
projection (its heads' 512 rows of w_out). Host sums the two partials per
batch and adds b_out.

On-device layout strategy (no on-device transposes anywhere):
  - Host supplies x^T (plus a ones row for the bias fold-in).
  - Q,K are produced transposed ([cols, T]) by making w the stationary
    matmul operand; V is produced normally ([T, cols]) by making x^T the
    stationary operand.
  - Scores are computed as S^T = K_tile^T.T @ Q^T (layout [Tk, Tq]), so
    exp(S^T) is directly the P^T the PV matmul needs as its moving operand.
  - No row-max subtraction: scores are ~N(0, 1/9) by construction, so
    exp never overflows; masked positions are zeroed multiplicatively
    (gpsimd affine_select) after exp.
  - Softmax denominators come free from a ones column interleaved into V
    (row 64 of the PV accumulator); normalization is a reciprocal plus a
    K=1 ones-broadcast matmul and one vector multiply.
  - All matmuls run in float32r (TF32-like; ~1.6e-4 relative error per
    matmul, measured) at 1 PE cycle/row.

Device output is the transposed partial projection out^T [1024, 2048];
host adds partials, bias, and un-transposes.
"""

import numpy as np

B, T, C = 4, 2048, 1024
H, D = 16, 64
HL = 8          # heads per core
HP = HL // 2    # head-pairs per core (row-packed in the PE array)
KCH = C // 128  # 8 full contraction chunks (plus 1-row bias chunk)
TCH = T // 512  # 4 T chunks of 512
SCALE = 1.0 / 8.0  # 1/sqrt(D)

_CACHE = {}


class _SkipPhase(Exception):
    pass


def _build(nrep=1, do_proj=True, do_attn=True):
    import concourse.bass as bass  # noqa: F401
    import concourse.mybir as mybir
    import concourse.tile as tile
    from concourse import bacc

    f32 = mybir.dt.float32
    f32r = mybir.dt.float32r
    Exp = mybir.ActivationFunctionType.Exp

    nc = bacc.Bacc("TRN2", target_bir_lowering=False, debug=False, num_devices=8)

    xt_d = nc.dram_tensor("xt", [C + 1, T], f32r, kind="ExternalInput")
    wqk_d = nc.dram_tensor("wqk", [C + 1, 1024], f32r, kind="ExternalInput")
    wv_d = nc.dram_tensor("wv", [C + 1, 512], f32r, kind="ExternalInput")
    wo_d = nc.dram_tensor("wo", [512, 1024], f32r, kind="ExternalInput")
    out_d = nc.dram_tensor("outT", [1024, T], f32, kind="ExternalOutput")
    # tiny pass-through tensor so a profiler can chain iterations sequentially
    chain_i = nc.dram_tensor("chain", [1, 8], f32, kind="ExternalInput")
    chain_o = nc.dram_tensor("chain_out", [1, 8], f32, kind="ExternalOutput")

    with tile.TileContext(nc) as tc, nc.allow_low_precision(
        reason="float32r (tf32) matmul operand production"
    ):
        import contextlib

        def _emit_body(ctx):
            # ---- long-lived pools ----
            qt_pool = ctx.enter_context(tc.tile_pool(name="qt", bufs=HP))
            kt_pool = ctx.enter_context(tc.tile_pool(name="kt", bufs=HP))
            v_pool = ctx.enter_context(tc.tile_pool(name="v", bufs=16))
            yt_pool = ctx.enter_context(tc.tile_pool(name="yt", bufs=HP))
            p_pool = ctx.enter_context(tc.tile_pool(name="p", bufs=6))
            misc_pool = ctx.enter_context(tc.tile_pool(name="misc", bufs=1))
            rcp_pool = ctx.enter_context(tc.tile_pool(name="rcp", bufs=3))

            qt = [qt_pool.tile([128, T], f32r, tag="qt", name="qt") for _ in range(HP)]
            kt = [kt_pool.tile([128, T], f32r, tag="kt", name="kt") for _ in range(HP)]
            v_sb = [v_pool.tile([128, 520], f32r, tag="v", name="v") for _ in range(16)]
            yt = [yt_pool.tile([128, T], f32r, tag="yt", name="yt") for _ in range(HP)]

            # ones helpers: [1,64] f32r for the denominator broadcast matmul,
            # [128,8] f32 source for V's interleaved ones columns.
            ones32 = misc_pool.tile([128, 8], f32, tag="ones32")
            nc.vector.memset(ones32[:], 1.0)
            chn = misc_pool.tile([1, 8], f32, tag="chn")
            nc.sync.dma_start(out=chn[:], in_=chain_i[:])
            nc.sync.dma_start(out=chain_o[:], in_=chn[:])

            if not do_proj:
                # attn-only probe: fill qt/kt/v with arbitrary DRAM data
                for i_, t_ in enumerate(qt + kt):
                    nc.sync.dma_start(out=t_[:], in_=xt_d[(i_ % 8) * 128 : (i_ % 8) * 128 + 128, :])
                for i_, t_ in enumerate(v_sb):
                    nc.sync.dma_start(out=t_[:], in_=xt_d[(i_ % 8) * 128 : (i_ % 8) * 128 + 128, 0:520])
            # ---- phase 1: projections (x^T and weights streamed by T-chunk) ----
            try:
              with (
                tc.tile_pool(name="xt", bufs=10) as xt_pool,
                tc.tile_pool(name="wqk", bufs=3) as wqk_pool,
                tc.tile_pool(name="wv", bufs=3) as wv_pool,
                tc.tile_pool(name="ps_pj", bufs=8, space="PSUM") as ps_pj,
            ):
                if not do_proj:
                    raise _SkipPhase
                wq9 = wqk_pool.tile([1, 1024], f32r, tag="wqk9", bufs=1)
                nc.sync.dma_start(out=wq9[:], in_=wqk_d[C : C + 1, :])
                wv9 = wv_pool.tile([1, 512], f32r, tag="wv9", bufs=1)
                nc.sync.dma_start(out=wv9[:], in_=wv_d[C : C + 1, :])
                for tch in range(TCH):
                    ts = tch * 512
                    xtc = [xt_pool.tile([128, 512], f32r, tag="xt", name="xt") for _ in range(KCH)]
                    xt9 = xt_pool.tile([1, 512], f32r, tag="xt9", bufs=2)
                    for k in range(KCH):
                        nc.sync.dma_start(
                            out=xtc[k][:],
                            in_=xt_d[k * 128 : (k + 1) * 128, ts : ts + 512],
                        )
                    nc.sync.dma_start(out=xt9[:], in_=xt_d[C : C + 1, ts : ts + 512])

                    # Q^T and K^T: stationary w, moving x^T  -> [cols, T]
                    # k-outer so weight tiles can stream through a small pool
                    ps_qk = [ps_pj.tile([128, 512], f32, tag="pj", name="pj") for _ in range(8)]
                    for k in range(KCH):
                        wq = wqk_pool.tile([128, 1024], f32r, tag="wqk", name="wqk")
                        nc.sync.dma_start(out=wq[:], in_=wqk_d[k * 128 : (k + 1) * 128, :])
                        for ct in range(8):
                            nc.tensor.matmul(
                                ps_qk[ct][:],
                                wq[:, ct * 128 : (ct + 1) * 128],
                                xtc[k][:],
                                start=(k == 0),
                                stop=False,
                            )
                    for ct in range(8):
                        nc.tensor.matmul(
                            ps_qk[ct][:],
                            wq9[:, ct * 128 : (ct + 1) * 128],
                            xt9[:],
                            start=False,
                            stop=True,
                        )
                        dst = qt[ct] if ct < HP else kt[ct - HP]
                        nc.vector.tensor_copy(dst[:, ts : ts + 512], ps_qk[ct][:])

                    # V: stationary x^T, moving w_v  -> [T, cols]
                    ps_v = [ps_pj.tile([128, 512], f32, tag="pj", name="pj") for _ in range(4)]
                    for k in range(KCH):
                        wvk = wv_pool.tile([128, 512], f32r, tag="wv", name="wv")
                        nc.sync.dma_start(out=wvk[:], in_=wv_d[k * 128 : (k + 1) * 128, :])
                        for tl in range(4):
                            nc.tensor.matmul(
                                ps_v[tl][:],
                                xtc[k][:, tl * 128 : (tl + 1) * 128],
                                wvk[:],
                                start=(k == 0),
                                stop=False,
                            )
                    for tl in range(4):
                        tt = tch * 4 + tl
                        nc.tensor.matmul(
                            ps_v[tl][:],
                            xt9[:, tl * 128 : (tl + 1) * 128],
                            wv9[:],
                            start=False,
                            stop=True,
                        )
                        # interleave into [h*65 .. h*65+64) + ones col at h*65+64
                        vt = v_sb[tt]
                        v_view = vt[:].rearrange("p (h c) -> p h c", c=65)
                        nc.vector.tensor_copy(
                            v_view[:, :, 0:64],
                            ps_v[tl][:].rearrange("p (h c) -> p h c", c=64),
                        )
                        nc.vector.tensor_copy(v_view[:, :, 64:65], ones32[:].unsqueeze(2))
            except _SkipPhase:
                pass

            if not do_attn:
                # proj-only probe: store qt as the output
                for i_ in range(4):
                    nc.sync.dma_start(out=out_d[i_ * 256 : i_ * 256 + 128, :], in_=qt[i_][:].bitcast(f32))
            # ---- phase 2: attention + output projection ----
            try:
              with (
                tc.tile_pool(name="wo", bufs=4) as wo_pool,
                tc.tile_pool(name="ostage", bufs=3) as ostage_pool,
            ):
                if not do_attn:
                    raise _SkipPhase
                wo = [wo_pool.tile([128, 1024], f32r, tag="wo", name="wo") for _ in range(4)]
                for k in range(4):
                    nc.sync.dma_start(out=wo[k][:], in_=wo_d[k * 128 : (k + 1) * 128, :])

                att_pools = tc.tile_pool(name="ps_s", bufs=4, space="PSUM"), tc.tile_pool(
                    name="ps_y", bufs=4, space="PSUM"
                )
                ps_s, ps_y = (att_pools[0].__enter__(), att_pools[1].__enter__())
                for hp in range(HP):
                    for qc in range(TCH):
                        qs = qc * 512
                        n_kt = 4 * (qc + 1)
                        ya = ps_y.tile([128, 512], f32, tag="y", name="ya")
                        yb = ps_y.tile([128, 512], f32, tag="y", name="yb")
                        for kti in range(n_kt):
                            ks = kti * 128
                            sa = ps_s.tile([128, 512], f32, tag="s", name="sa")
                            sb_ = ps_s.tile([128, 512], f32, tag="s", name="sb")
                            nc.tensor.matmul(
                                sa[:],
                                kt[hp][0:64, ks : ks + 128],
                                qt[hp][0:64, qs : qs + 512],
                                start=True,
                                stop=True,
                                tile_position=(0, 0),
                            )
                            nc.tensor.matmul(
                                sb_[:],
                                kt[hp][64:128, ks : ks + 128],
                                qt[hp][64:128, qs : qs + 512],
                                start=True,
                                stop=True,
                                tile_position=(64, 0),
                            )
                            pa = p_pool.tile([128, 512], f32r, tag="p", name="pa")
                            pb = p_pool.tile([128, 512], f32r, tag="p", name="pb")
                            nc.scalar.activation(pa[:], sa[:], Exp, scale=SCALE)
                            nc.scalar.activation(pb[:], sb_[:], Exp, scale=SCALE)
                            d = ks - qs
                            if d >= 0:  # diagonal-crossing tile: zero invalid
                                for p_t in (pa, pb):
                                    nc.gpsimd.affine_select(
                                        out=p_t[:],
                                        in_=p_t[:],
                                        compare_op=mybir.AluOpType.is_ge,
                                        fill=0.0,
                                        base=-d,
                                        pattern=[[1, 512]],
                                        channel_multiplier=-1,
                                    )
                            ha, hb = 2 * hp, 2 * hp + 1
                            nc.tensor.matmul(
                                ya[0:65, :],
                                v_sb[kti][:, ha * 65 : ha * 65 + 65],
                                pa[:],
                                start=(kti == 0),
                                stop=(kti == n_kt - 1),
                            )
                            nc.tensor.matmul(
                                yb[0:65, :],
                                v_sb[kti][:, hb * 65 : hb * 65 + 65],
                                pb[:],
                                start=(kti == 0),
                                stop=(kti == n_kt - 1),
                            )
                        # normalize: y[0:64] / y[64]
                        for off, yy in ((0, ya), (64, yb)):
                            rcp = rcp_pool.tile([1, 512], f32, tag="rcp", name="rcp")
                            nc.vector.reciprocal(rcp[:], yy[64:65, :])
                            rbc = rcp_pool.tile([64, 512], f32, tag="rbc", name="rbc")
                            nc.gpsimd.partition_broadcast(rbc[:], rcp[:])
                            nc.vector.tensor_mul(
                                yt[hp][off : off + 64, qs : qs + 512],
                                yy[0:64, :],
                                rbc[:],
                            )

                att_pools[1].__exit__(None, None, None)
                att_pools[0].__exit__(None, None, None)

                # output projection: out^T[ct, tq] = wo.T @ y^T
                with tc.tile_pool(name="ps_o", bufs=4, space="PSUM") as ps_o:
                    for ct in range(8):
                        for qc in range(TCH):
                            qs = qc * 512
                            ps = ps_o.tile([128, 512], f32, tag="pso")
                            for k in range(4):
                                nc.tensor.matmul(
                                    ps[:],
                                    wo[k][:, ct * 128 : (ct + 1) * 128],
                                    yt[k][:, qs : qs + 512],
                                    start=(k == 0),
                                    stop=(k == 3),
                                )
                            st = ostage_pool.tile([128, 512], f32, tag="ost", name="ost")
                            nc.vector.tensor_copy(st[:], ps[:])
                            nc.sync.dma_start(
                                out=out_d[ct * 128 : (ct + 1) * 128, qs : qs + 512],
                                in_=st[:],
                            )
            except _SkipPhase:
                pass

        if nrep == 1:
            with contextlib.ExitStack() as ctx:
                _emit_body(ctx)
        else:
            with tc.For_i(0, nrep, 1):
                with contextlib.ExitStack() as ctx:
                    _emit_body(ctx)

    nc.compile()
    return nc


def _get_nc():
    if "nc" not in _CACHE:
        _CACHE["nc"] = _build()
    return _CACHE["nc"]


def kernel(x, w_qkv, b_qkv, w_out, b_out):
    from concourse.bass_utils import run_bass_kernel_spmd

    x = np.asarray(x, dtype=np.float32)
    w_qkv = np.asarray(w_qkv, dtype=np.float32)
    b_qkv = np.asarray(b_qkv, dtype=np.float32)
    w_out = np.asarray(w_out, dtype=np.float32)
    b_out = np.asarray(b_out, dtype=np.float32)

    in_maps = []
    for core in range(8):
        b = core // 2
        hg = core % 2
        cs = hg * 512  # column offset of this core's heads within each block
        xt = np.empty((C + 1, T), dtype=np.float32)
        xt[:C] = x[b].T
        xt[C] = 1.0
        wqk = np.empty((C + 1, 1024), dtype=np.float32)
        wqk[:C, 0:512] = w_qkv[:, cs : cs + 512]              # Q cols
        wqk[:C, 512:1024] = w_qkv[:, C + cs : C + cs + 512]   # K cols
        wqk[C, 0:512] = b_qkv[cs : cs + 512]
        wqk[C, 512:1024] = b_qkv[C + cs : C + cs + 512]
        wv = np.empty((C + 1, 512), dtype=np.float32)
        wv[:C] = w_qkv[:, 2 * C + cs : 2 * C + cs + 512]
        wv[C] = b_qkv[2 * C + cs : 2 * C + cs + 512]
        wo = np.ascontiguousarray(w_out[cs : cs + 512, :])
        in_maps.append(
            {
                "xt": np.ascontiguousarray(xt),
                "wqk": np.ascontiguousarray(wqk),
                "wv": wv,
                "wo": wo,
                "chain": np.zeros((1, 8), np.float32),
            }
        )

    _CACHE["in_maps"] = in_maps
    res = run_bass_kernel_spmd(_get_nc(), in_maps, core_ids=list(range(8)))

    out = np.empty((B, T, C), dtype=np.float32)
    for b in range(B):
        acc = res.results[2 * b]["outT"] + res.results[2 * b + 1]["outT"]
        out[b] = acc.T + b_out[None, :]
    return out



# revision 8
# speedup vs baseline: 1.8314x; 1.8314x over previous
"""Causal self-attention Bass/Tile kernel for 8 Trainium2 NeuronCores.

Problem: B=4, T=2048, C=1024, H=16, D=64 (fp32).
  qkv = x @ w_qkv + b_qkv ; causal softmax attention ; y @ w_out + b_out

Sharding (8 cores): core i handles batch b = i//2 and head-group hg = i%2
(8 of the 16 heads). Each core computes Q/K/V projections restricted to its
heads, full causal attention for those heads, and a partial output
projection (its heads' 512 rows of w_out). Host sums the two partials per
batch and adds the effective bias.

Design notes (v2):
  - All matmul operands are bf16 (PSUM accumulates fp32); tolerance is
    2e-2 and measured error is ~1e-3. Halves SBUF/DMA and enables DVE
    2x modes + FWL weight loads.
  - Weights are DMA'd once and stay in SBUF.
  - qkv biases never touch the PE: b_q is added during the Q PSUM-evict
    (per-partition tensor_scalar), b_k is dropped (shifts every logit of
    a query equally -> softmax invariant), b_v is folded into the host
    bias (y rows of softmax sum to 1, so + b_v passes through attention;
    host adds b_v @ w_out to the output bias).
  - Scores are computed transposed (S^T[k,q]) with two row-tiled K=64
    matmuls (two heads) into one 2-bank PSUM tile [128,1024]; ONE ACT
    exp instruction covers both heads.
  - Causal mask: memset of the fully-masked strip + one bf16 tensor_mul
    against a precomputed [128,128] lower-triangle mask on the single
    boundary sub-block (the mask pattern is shift-invariant).
  - Softmax denominators ride along as a ones-column interleaved in V
    (row 64 of the PV accumulator). Normalization = reciprocal_approx_fast
    + gpsimd partition_broadcast + one fused scalar_tensor_tensor that
    reads PSUM and writes the normalized bf16 y^T.
  - Emission is software-pipelined: projection matmuls for T-chunk tch+1
    and output-projection matmuls for chunk qc-1 are interleaved into the
    attention stream so the PE queue never drains (keeps HAM at 2.4 GHz)
    and exp latency is hidden.

Device output is the transposed partial projection out^T [1024, 2048];
host adds partials + effective bias and un-transposes.
"""

import collections
import contextlib

import numpy as np

B, T, C = 4, 2048, 1024
H, D = 16, 64
HL = 8          # heads per core
HP = HL // 2    # head-pairs per core (row-packed in the PE array)
KCH = C // 128  # 8 contraction chunks
TCH = T // 512  # 4 T chunks of 512
SCALE = 1.0 / 8.0  # 1/sqrt(D)

_CACHE = {}


def _build(dump=False):
    import concourse.bass as bass  # noqa: F401
    import concourse.mybir as mybir
    import concourse.tile as tile
    from concourse import bacc

    f32 = mybir.dt.float32
    bf16 = mybir.dt.bfloat16
    Exp = mybir.ActivationFunctionType.Exp
    Mul = mybir.AluOpType.mult

    nc = bacc.Bacc("TRN2", target_bir_lowering=False, debug=False, num_devices=8)

    xt_d = nc.dram_tensor("xt", [C, T], bf16, kind="ExternalInput")
    wqk_d = nc.dram_tensor("wqk", [C, 1024], bf16, kind="ExternalInput")
    wv_d = nc.dram_tensor("wv", [C, 512], bf16, kind="ExternalInput")
    wo_d = nc.dram_tensor("wo", [512, 1024], bf16, kind="ExternalInput")
    qb_d = nc.dram_tensor("qb", [128, 4], f32, kind="ExternalInput")
    mask_d = nc.dram_tensor("mask", [128, 128], bf16, kind="ExternalInput")
    out_d = nc.dram_tensor("outT", [1024, T], f32, kind="ExternalOutput")
    # tiny pass-through tensor so a profiler can chain iterations sequentially
    chain_i = nc.dram_tensor("chain", [1, 8], f32, kind="ExternalInput")
    chain_o = nc.dram_tensor("chain_out", [1, 8], f32, kind="ExternalOutput")
    if dump:
        dbg_qt = nc.dram_tensor("dbg_qt", [128, T], bf16, kind="ExternalOutput")
        dbg_kt = nc.dram_tensor("dbg_kt", [128, T], bf16, kind="ExternalOutput")
        dbg_v = nc.dram_tensor("dbg_v", [128, 520], bf16, kind="ExternalOutput")
        dbg_s = nc.dram_tensor("dbg_s", [128, 4096], f32, kind="ExternalOutput")
        dbg_p = nc.dram_tensor("dbg_p", [128, 4096], bf16, kind="ExternalOutput")
        dbg_y2 = nc.dram_tensor("dbg_y2", [128, 1024], f32, kind="ExternalOutput")
        dbg_yt = nc.dram_tensor("dbg_yt", [128, T], bf16, kind="ExternalOutput")
        dbg_rcp = nc.dram_tensor("dbg_rcp", [2, 512], f32, kind="ExternalOutput")
        dbg_rbc = nc.dram_tensor("dbg_rbc", [128, 1024], f32, kind="ExternalOutput")

    with (
        tile.TileContext(nc) as tc,
        nc.allow_low_precision(reason="bf16 attention pipeline"),
        contextlib.ExitStack() as ctx,
    ):
        # ---- long-lived SBUF pools ----
        wqk_pool = ctx.enter_context(tc.tile_pool(name="wqk", bufs=KCH))
        wv_pool = ctx.enter_context(tc.tile_pool(name="wv", bufs=KCH))
        wo_pool = ctx.enter_context(tc.tile_pool(name="wo", bufs=4))
        qt_pool = ctx.enter_context(tc.tile_pool(name="qt", bufs=HP))
        kt_pool = ctx.enter_context(tc.tile_pool(name="kt", bufs=HP))
        v_pool = ctx.enter_context(tc.tile_pool(name="v", bufs=16))
        yt_pool = ctx.enter_context(tc.tile_pool(name="yt", bufs=HP))
        misc_pool = ctx.enter_context(tc.tile_pool(name="misc", bufs=1))
        xt_pool = ctx.enter_context(tc.tile_pool(name="xt", bufs=16))
        p_pool = ctx.enter_context(tc.tile_pool(name="p", bufs=4))
        rcp_pool = ctx.enter_context(tc.tile_pool(name="rcp", bufs=4))
        rbc_pool = ctx.enter_context(tc.tile_pool(name="rbc", bufs=4))
        ost_pool = ctx.enter_context(tc.tile_pool(name="ost", bufs=3))
        # ---- PSUM: pj 2 banks + s 2x2 banks + y 1x2 banks = 8 banks ----
        ps_pj = ctx.enter_context(tc.tile_pool(name="ps_pj", bufs=2, space="PSUM"))
        ps_s = ctx.enter_context(tc.tile_pool(name="ps_s", bufs=2, space="PSUM"))
        ps_y = ctx.enter_context(tc.tile_pool(name="ps_y", bufs=1, space="PSUM"))

        wqk_sb = [wqk_pool.tile([128, 1024], bf16, tag="wqk", name="wqk") for _ in range(KCH)]
        wv_sb = [wv_pool.tile([128, 512], bf16, tag="wv", name="wv") for _ in range(KCH)]
        wo_sb = [wo_pool.tile([128, 1024], bf16, tag="wo", name="wo") for _ in range(4)]
        qt = [qt_pool.tile([128, T], bf16, tag="qt", name="qt") for _ in range(HP)]
        kt = [kt_pool.tile([128, T], bf16, tag="kt", name="kt") for _ in range(HP)]
        v_sb = [v_pool.tile([128, 520], bf16, tag="v", name="v") for _ in range(16)]
        yt = [yt_pool.tile([128, T], bf16, tag="yt", name="yt") for _ in range(HP)]
        qb_sb = misc_pool.tile([128, 4], f32, tag="qb")
        mask_sb = misc_pool.tile([128, 128], bf16, tag="mask")
        chn = misc_pool.tile([1, 8], f32, tag="chn")

        def init_ops():
            nc.sync.dma_start(out=chn[:], in_=chain_i[:])
            nc.sync.dma_start(out=chain_o[:], in_=chn[:])
            nc.sync.dma_start(out=qb_sb[:], in_=qb_d[:])
            nc.sync.dma_start(out=mask_sb[:], in_=mask_d[:])
            for k in range(KCH):
                nc.sync.dma_start(out=wqk_sb[k][:], in_=wqk_d[k * 128 : (k + 1) * 128, :])
                nc.sync.dma_start(out=wv_sb[k][:], in_=wv_d[k * 128 : (k + 1) * 128, :])
            for k in range(4):
                nc.sync.dma_start(out=wo_sb[k][:], in_=wo_d[k * 128 : (k + 1) * 128, :])
            # ones columns interleaved into V (denominator rows of PV)
            for tt in range(16):
                v_view = v_sb[tt][:].rearrange("p (h c) -> p h c", c=65)
                nc.vector.memset(v_view[:, :, 64:65], 1.0)

        def proj_steps(tch):
            """QKV projection for T-chunk tch. Yields after each PSUM-tile's
            worth of matmuls (~8 MMs) so the driver can interleave."""
            ts = tch * 512
            xtc = [xt_pool.tile([128, 512], bf16, tag="xt", name="xt") for _ in range(KCH)]
            for k in range(KCH):
                nc.sync.dma_start(
                    out=xtc[k][:], in_=xt_d[k * 128 : (k + 1) * 128, ts : ts + 512]
                )
            # Q^T, K^T: stationary w chunk, moving x^T -> [cols, 512]
            for ct in range(8):
                ps = ps_pj.tile([128, 512], f32, tag="pj", name="pjq")
                for k in range(KCH):
                    nc.tensor.matmul(
                        ps[:],
                        wqk_sb[k][:, ct * 128 : (ct + 1) * 128],
                        xtc[k][:],
                        start=(k == 0),
                        stop=(k == KCH - 1),
                    )
                if ct < 4:
                    nc.vector.tensor_scalar_add(
                        qt[ct][:, ts : ts + 512], ps[:], qb_sb[:, ct : ct + 1]
                    )
                else:
                    nc.vector.tensor_copy(kt[ct - 4][:, ts : ts + 512], ps[:])
                yield
            # V: stationary x^T slice, moving w_v -> [128 rows of T, 512]
            for tl in range(4):
                ps = ps_pj.tile([128, 512], f32, tag="pj", name="pjv")
                for k in range(KCH):
                    nc.tensor.matmul(
                        ps[:],
                        xtc[k][:, tl * 128 : (tl + 1) * 128],
                        wv_sb[k][:],
                        start=(k == 0),
                        stop=(k == KCH - 1),
                    )
                vt = v_sb[tch * 4 + tl]
                v_view = vt[:].rearrange("p (h c) -> p h c", c=65)
                nc.vector.tensor_copy(
                    v_view[:, :, 0:64],
                    ps[:].rearrange("p (h c) -> p h c", c=64),
                )
                yield

        def attn_steps(qc):
            """Attention for query chunk qc, all head-pairs. Yields per
            kt-iteration and per normalization half."""
            qs = qc * 512
            n_kt = 4 * (qc + 1)
            for hp in range(HP):
                ha, hb = 2 * hp, 2 * hp + 1
                y2 = ps_y.tile([128, 1024], f32, tag="y", name="y2")
                for kti in range(n_kt):
                    ks = kti * 128
                    s2 = ps_s.tile([128, 1024], f32, tag="s", name="s2")
                    nc.tensor.matmul(
                        s2[:, 0:512],
                        kt[hp][0:64, ks : ks + 128],
                        qt[hp][0:64, qs : qs + 512],
                        start=True,
                        stop=True,
                        tile_position=(0, 0),
                    )
                    nc.tensor.matmul(
                        s2[:, 512:1024],
                        kt[hp][64:128, ks : ks + 128],
                        qt[hp][64:128, qs : qs + 512],
                        start=True,
                        stop=True,
                        tile_position=(64, 0),
                    )
                    p2 = p_pool.tile([128, 1024], bf16, tag="p", name="p2")
                    if dump and qc == 0 and hp == 0:
                        sst = ost_pool.tile([128, 512], f32, tag="ost", name="sdmp")
                        nc.vector.tensor_copy(sst[:], s2[:, 0:512])
                        nc.sync.dma_start(
                            out=dbg_s[:, kti * 1024 : kti * 1024 + 512], in_=sst[:]
                        )
                        sst2 = ost_pool.tile([128, 512], f32, tag="ost", name="sdmp2")
                        nc.vector.tensor_copy(sst2[:], s2[:, 512:1024])
                        nc.sync.dma_start(
                            out=dbg_s[:, kti * 1024 + 512 : (kti + 1) * 1024],
                            in_=sst2[:],
                        )
                    nc.scalar.activation(p2[:], s2[:], Exp, scale=SCALE)
                    j = kti - 4 * qc
                    if j >= 0:  # diagonal-crossing tile
                        d = 128 * j
                        for off in (0, 512):
                            if d > 0:
                                nc.vector.memset(p2[:, off : off + d], 0.0)
                            blk = p2[:, off + d : off + d + 128]
                            nc.vector.tensor_mul(blk, blk, mask_sb[:])
                    if dump and qc == 0 and hp == 0:
                        nc.sync.dma_start(
                            out=dbg_p[:, kti * 1024 : (kti + 1) * 1024], in_=p2[:]
                        )
                    nc.tensor.matmul(
                        y2[0:65, 0:512],
                        v_sb[kti][:, ha * 65 : ha * 65 + 65],
                        p2[:, 0:512],
                        start=(kti == 0),
                        stop=(kti == n_kt - 1),
                    )
                    nc.tensor.matmul(
                        y2[0:65, 512:1024],
                        v_sb[kti][:, hb * 65 : hb * 65 + 65],
                        p2[:, 512:1024],
                        start=(kti == 0),
                        stop=(kti == n_kt - 1),
                    )
                    yield
                if dump and qc == 0 and hp == 0:
                    y2st = ost_pool.tile([128, 512], f32, tag="ost", name="y2dmp")
                    nc.vector.tensor_copy(y2st[:], y2[:, 0:512])
                    nc.sync.dma_start(out=dbg_y2[:, 0:512], in_=y2st[:])
                    y2st2 = ost_pool.tile([128, 512], f32, tag="ost", name="y2dmp2")
                    nc.vector.tensor_copy(y2st2[:], y2[:, 512:1024])
                    nc.sync.dma_start(out=dbg_y2[:, 512:1024], in_=y2st2[:])
                # normalize: yt = y2[0:64] / y2[64]
                for h, off in ((0, 0), (1, 512)):
                    lr = rcp_pool.tile([1, 512], f32, tag="lr", name="lr")
                    nc.vector.tensor_copy(lr[:], y2[64:65, off : off + 512])
                    rcp = rcp_pool.tile([1, 512], f32, tag="rcp", name="rcp")
                    nc.vector.reciprocal_approx_fast(rcp[:], lr[:])
                    rbc = rbc_pool.tile([64, 512], f32, tag="rbc", name="rbc")
                    nc.gpsimd.partition_broadcast(rbc[:], rcp[:])
                    if dump and qc == 0 and hp == 0:
                        nc.sync.dma_start(out=dbg_rcp[h : h + 1, :], in_=rcp[:])
                        nc.sync.dma_start(
                            out=dbg_rbc[0:64, off : off + 512], in_=rbc[:]
                        )
                    nc.vector.scalar_tensor_tensor(
                        out=yt[hp][h * 64 : (h + 1) * 64, qs : qs + 512],
                        in0=y2[0:64, off : off + 512],
                        scalar=1.0,
                        in1=rbc[:],
                        op0=Mul,
                        op1=Mul,
                    )
                    yield

        def oproj_steps(qc):
            """Output projection for query chunk qc: out^T = wo^T @ y^T."""
            qs = qc * 512
            for ct in range(8):
                ps = ps_pj.tile([128, 512], f32, tag="pj", name="pjo")
                for k in range(4):
                    nc.tensor.matmul(
                        ps[:],
                        wo_sb[k][:, ct * 128 : (ct + 1) * 128],
                        yt[k][:, qs : qs + 512],
                        start=(k == 0),
                        stop=(k == 3),
                    )
                st = ost_pool.tile([128, 512], f32, tag="ost", name="ost")
                nc.vector.tensor_copy(st[:], ps[:])
                nc.sync.dma_start(
                    out=out_d[ct * 128 : (ct + 1) * 128, qs : qs + 512], in_=st[:]
                )
                yield

        # ---- driver: interleave proj/oproj into the attention stream ----
        init_ops()
        for _ in proj_steps(0):
            pass
        pending = collections.deque()
        for qc in range(TCH):
            if qc + 1 < TCH:
                pending.append(proj_steps(qc + 1))
            n_yields = HP * (4 * (qc + 1) + 2)
            n_pending = (12 if qc + 1 < TCH else 0) + (8 if qc >= 1 else 0)
            done = 0
            for yi, _ in enumerate(attn_steps(qc)):
                while pending and done < (yi + 1) * n_pending / n_yields:
                    try:
                        next(pending[0])
                        done += 1
                    except StopIteration:
                        pending.popleft()
            pending.append(oproj_steps(qc))
        while pending:
            try:
                next(pending[0])
            except StopIteration:
                pending.popleft()

        if dump:
            nc.sync.dma_start(out=dbg_qt[:], in_=qt[0][:])
            nc.sync.dma_start(out=dbg_kt[:], in_=kt[0][:])
            nc.sync.dma_start(out=dbg_v[:], in_=v_sb[0][:])
            nc.sync.dma_start(out=dbg_yt[:], in_=yt[0][:])

    nc.compile()
    return nc


def _get_nc():
    if "nc" not in _CACHE:
        _CACHE["nc"] = _build()
    return _CACHE["nc"]


def kernel(x, w_qkv, b_qkv, w_out, b_out):
    import ml_dtypes
    from concourse.bass_utils import run_bass_kernel_spmd

    bf16 = ml_dtypes.bfloat16
    x = np.asarray(x, dtype=np.float32)
    w_qkv = np.asarray(w_qkv, dtype=np.float32)
    b_qkv = np.asarray(b_qkv, dtype=np.float32)
    w_out = np.asarray(w_out, dtype=np.float32)
    b_out = np.asarray(b_out, dtype=np.float32)

    mask = np.triu(np.ones((128, 128), dtype=np.float32)).astype(bf16)
    in_maps = []
    for core in range(8):
        b = core // 2
        hg = core % 2
        cs = hg * 512  # column offset of this core's heads within each block
        wqk = np.empty((C, 1024), dtype=bf16)
        wqk[:, 0:512] = w_qkv[:, cs : cs + 512]              # Q cols
        wqk[:, 512:1024] = w_qkv[:, C + cs : C + cs + 512]   # K cols
        in_maps.append(
            {
                "xt": np.ascontiguousarray(x[b].T).astype(bf16),
                "wqk": wqk,
                "wv": w_qkv[:, 2 * C + cs : 2 * C + cs + 512].astype(bf16),
                "wo": w_out[cs : cs + 512, :].astype(bf16),
                "qb": np.ascontiguousarray(
                    b_qkv[cs : cs + 512].reshape(4, 128).T
                ).astype(np.float32),
                "mask": mask,
                "chain": np.zeros((1, 8), np.float32),
            }
        )

    _CACHE["in_maps"] = in_maps
    res = run_bass_kernel_spmd(_get_nc(), in_maps, core_ids=list(range(8)))

    # effective bias: b_out + V-bias pushed through the output projection
    bias_eff = (b_out + b_qkv[2 * C :] @ w_out).astype(np.float32)
    out = np.empty((B, T, C), dtype=np.float32)
    for b in range(B):
        acc = res.results[2 * b]["outT"] + res.results[2 * b + 1]["outT"]
        out[b] = acc.T + bias_eff[None, :]
    return out
